# revision 20
# baseline (speedup 1.0000x reference)
"""MemNN layer kernel for 8 Trainium2 NeuronCores.

Strategy (batch-sharded, 16 batches/core):
- The 4 embedding tables are interleaved into one "megatable" whose row v is
  [A0hi|A0lo|A1hi|A1lo|A2hi|A2lo|A3hi|A3lo] (8 x 128 fp16 = 2048 B), where
  hi = fp16(A), lo = fp16(A - hi).  One dma_gather fetches all 4 tables for a
  token at f32-exact precision (hi+lo), at the same GpSimd descriptor-gen cost
  as a single-table gather (cost is per-index, not per-byte).
- dma_gather in transpose mode lands embeddings with embd on partitions:
  G[p, slice, i] = megarow(idx_i)[slice*128+p].  int16 gather indices only
  reach 32767, so two passes per chunk: lo pass (rows < 32768, sentinel row 0
  which is all-zero padding) and hi pass (rows >= 32768 rebased, sentinel ->
  appended all-zero row 50000).
- 20-token sentence sums: DVE tensor_reduce over a 5-D AP that folds the
  lo/hi passes and the 20 tokens in one op -> S[128e, 8slices, 800slots] f32.
- Hops run in embd-on-partition layout: logits via elementwise mul + ones-
  matmul partition reduce, softmax on [1, 800], p broadcast via e0-selector
  matmul, weighted c-sum via DVE reduce.
- Final projection out[v, b] = sum_e A3[v, e] u[e, b]: A3^T fp16 is streamed
  from DRAM (pre-transposed on host) as PE lhsT per 128-vocab chunk, rhs =
  fp16(u); PSUM -> SBUF -> one contiguous store in [128, 391, 16] layout that
  the host rearranges.
"""

import numpy as np

HOPS = 3
VOCAB = 50000
EMBD = 128
BS = 128
STORY = 50
SENT = 20
QLEN = 20
NCORES = 8
BSH = BS // NCORES          # 16 batches per core
SLOTS = BSH * STORY         # 800 (b, s) slots per core
SPLIT = 32768               # int16-reachable rows per gather base
ZROW = VOCAB                # appended all-zero megatable row
HI_SENT = ZROW - SPLIT      # 17232: hi-pass sentinel (-> zero row)
NROWS = VOCAB + 1           # 50001
CHUNKS = [1280] * 12 + [640]  # %128==0, %20==0; 322 rx descs/lane so 3 overlap in the 1024 ring
GBUFS = 3                     # gather tile-pool depth
PREP_NQ = 0                   # >0: prepare_only desc-gen on PREP_NQ queues, serialized triggers
QPAD = 24                   # per-batch query tokens padded 20 -> 24
QIDX = BSH * QPAD           # 384 (%128 == 0)
VPAD = 50048                # vocab padded to 391*128 for projection
NVC = VPAD // 128           # 391 projection chunks

_cache = {}


def _wrap_idxs(lst):
    """int16 gather index layout: [128, n/16]; position i -> [i%16, i//16], tiled 8x."""
    a = np.asarray(lst).astype(np.int16).reshape(-1, 16).T.copy()
    return np.tile(a, (8, 1))


def _mk_ap(base_ap, dims, extra_offset_elems=0):
    """AP with the partition pair of base_ap and given free (stride, count) pairs."""
    import concourse.bass as bass
    ap = [tuple(base_ap.ap[0])] + [tuple(d) for d in dims]
    return bass.AP(base_ap.tensor, base_ap.offset + extra_offset_elems, ap)


def _build(scale, qmap=None):
    import concourse.tile as tile
    from concourse import bacc, mybir

    f32 = mybir.dt.float32
    i32 = mybir.dt.int32
    f16 = mybir.dt.float16
    i16 = mybir.dt.int16

    nq = PREP_NQ if PREP_NQ else (1 + max(qmap.values()) if qmap else 1)
    nc = bacc.Bacc("TRN2", target_bir_lowering=False, debug=False,
                   num_swdge_queues=nq)
    gather_names = []
    _gi = [0]

    def _gq():
        i = _gi[0]
        _gi[0] += 1
        return qmap.get(i, 0) if qmap else 0

    mega = nc.dram_tensor("mega", [NROWS, 512], i16, kind="ExternalInput")
    a3t = nc.dram_tensor("a3t", [128, VPAD], f16, kind="ExternalInput")
    ilo = nc.dram_tensor("ilo", [128, SLOTS * SENT // 16], i16, kind="ExternalInput")
    iqlo = nc.dram_tensor("iqlo", [128, QIDX // 16], i16, kind="ExternalInput")
    tat = nc.dram_tensor("tat", [128, STORY], f32, kind="ExternalInput")
    tct = nc.dram_tensor("tct", [128, STORY], f32, kind="ExternalInput")
    out = nc.dram_tensor("outp", [128, NVC * BSH], f32, kind="ExternalOutput")

    with tile.TileContext(nc) as tc:
        with (
            tc.tile_pool(name="consts", bufs=1) as cpool,
            tc.tile_pool(name="sacc", bufs=1) as spool,
        ):
            # ---- index loads first: they gate the first gather
            t_ilo = cpool.tile([128, SLOTS * SENT // 16], i16, tag="ilo")
            nc.sync.dma_start(t_ilo[:], ilo[:])
            t_iqlo = cpool.tile([128, QIDX // 16], i16, tag="iqlo")
            nc.sync.dma_start(t_iqlo[:], iqlo[:])

            t_tat = cpool.tile([128, STORY], f32, tag="tat")
            nc.sync.dma_start(t_tat[:], tat[:])
            t_tct = cpool.tile([128, STORY], f32, tag="tct")
            nc.sync.dma_start(t_tct[:], tct[:])
            ones_col = cpool.tile([128, 1], f32, tag="ones_col")  # lhsT for partition sum
            nc.vector.memset(ones_col[:], 1.0)
            e0row = cpool.tile([128, 128], f16, tag="e0row")      # lhsT for p broadcast
            nc.vector.memset(e0row[:], 0.0)
            nc.vector.memset(e0row[0:1, :], 1.0)

            # ---- S accumulator [128, 8 slices, 800 slots] f32
            S = spool.tile([128, 4, SLOTS], i32, tag="S")
            uq = spool.tile([128, 4, BSH], f32, tag="uq")  # query-sum per slice

            # ---- gather + reduce phase (int16 megatable, 1024 B rows)
            # int32 accumulation of int16 values is exact; scale applied later
            low_prec = nc.allow_low_precision(reason="int32 accumulation of int16 is exact")
            low_prec.__enter__()
            # PREP_NQ mode: all desc-gen preps are emitted first (they run
            # concurrently on the Q7s, one ring per queue), then count=1
            # triggers fire the transfers strictly in chunk order -- each
            # trigger waits on its own prep (psem) and on the previous
            # chunk's DMA completion (gsem of that chunk's queue) so
            # transpose-gather DMA streams never interleave across queues.
            # Consumer reduces carry explicit DMA-completion waits (Tile's
            # auto-wiring for prepare_only gathers proved wrong on HW).
            ngath = len(CHUNKS) + 1
            all_chunks = list(CHUNKS) + [QIDX]
            if PREP_NQ:
                gq_of = [i % PREP_NQ for i in range(ngath)]
                qpos = []  # index of chunk within its queue
                _cnt = [0] * PREP_NQ
                for i in range(ngath):
                    qpos.append(_cnt[gq_of[i]])
                    _cnt[gq_of[i]] += 1
                gsems = [nc.alloc_semaphore(f"gdma{q}") for q in range(PREP_NQ)]
                psems = [nc.alloc_semaphore(f"gprep{q}") for q in range(PREP_NQ)]

            with tc.tile_pool(name="gath", bufs=(1 if PREP_NQ else GBUFS)) as gpool:
                tiles = []
                pos = 0
                for ci, ch in enumerate(CHUNKS):
                    g_lo = gpool.tile([128, 4, ch], i16, tag=f"g_lo{ci if PREP_NQ else ''}")
                    tiles.append(g_lo)
                    pos += ch
                gq_lo = gpool.tile([128, 4, QIDX], i16, tag="gq_lo")
                tiles.append(gq_lo)

                def idx_ap_of(ci):
                    if ci == len(CHUNKS):
                        return t_iqlo[:]
                    pos = sum(CHUNKS[:ci])
                    return t_ilo[:, pos // 16:(pos + CHUNKS[ci]) // 16]

                if PREP_NQ:
                    for ci, ch in enumerate(all_chunks):
                        qn = gq_of[ci]
                        prep = nc.gpsimd.dma_gather(
                            tiles[ci][:], mega[:], idx_ap_of(ci), ch, ch, 512,
                            transpose=True, single_packet=False,
                            queue_num=qn, prepare_only=True, sem=gsems[qn])
                        prep.then_inc(psems[qn], 1)
                        gather_names.append(prep.ins.name)
                    for ci in range(ngath):
                        qn = gq_of[ci]
                        if ci > 0:
                            pq = gq_of[ci - 1]
                            nc.gpsimd.wait_ge(gsems[pq], 16 * (qpos[ci - 1] + 1))
                        tr = nc.gpsimd.trigger_dma(count=1, queue_num=qn)
                        tr._wait_ge(psems[qn], qpos[ci] + 1)
                else:
                    for ci, ch in enumerate(all_chunks):
                        gather_names.append(nc.gpsimd.dma_gather(
                            tiles[ci][:], mega[:], idx_ap_of(ci), ch, ch, 512,
                            transpose=True, single_packet=False,
                            queue_num=_gq()).ins.name)

                pos = 0
                for ci, ch in enumerate(CHUNKS):
                    nslot = ch // SENT
                    s0 = pos // SENT
                    # DVE reduce over tokens [128, 4, nslot, 20] -> int32 (exact)
                    red_lo = _mk_ap(tiles[ci][:], [(ch, 4), (SENT, nslot), (1, SENT)])
                    if PREP_NQ:
                        nc.vector.wait_ge(gsems[gq_of[ci]], 16 * (qpos[ci] + 1))
                    nc.vector.tensor_reduce(
                        S[:, :, s0:s0 + nslot], red_lo,
                        mybir.AxisListType.X, mybir.AluOpType.add)
                    pos += ch

                # query-token sums: [128, 4, 16, 24] reduce X(24) -> uq
                q_lo_in = _mk_ap(gq_lo[:], [(QIDX, 4), (QPAD, BSH), (1, QPAD)])
                if PREP_NQ:
                    qi = ngath - 1
                    nc.vector.wait_ge(gsems[gq_of[qi]], 16 * (qpos[qi] + 1))
                nc.vector.tensor_reduce(
                    uq[:], q_lo_in, mybir.AxisListType.X, mybir.AluOpType.add)

            low_prec.__exit__(None, None, None)
            with (
                tc.tile_pool(name="hopp", bufs=1) as hpool,
                tc.tile_pool(name="psum", bufs=2, space="PSUM") as ppool,
            ):
                # u0 = scale * (q-sum of table 0)
                u = hpool.tile([128, BSH], f32, tag="u")
                nc.vector.tensor_scalar_mul(u[:], uq[:, 0, :], scale)

                t0 = hpool.tile([128, BSH, STORY], f32, tag="t0")
                pe_sb = hpool.tile([128, BSH, STORY], f16, tag="pe_sb")
                nc.vector.memset(pe_sb[:], 0.0)
                lg = hpool.tile([1, BSH, STORY], f32, tag="lg")
                red2 = hpool.tile([1, BSH], f32, tag="red2")
                red_u = hpool.tile([128, BSH], f32, tag="redu")

                def smv(k, off=0, nb=BSH):
                    return _mk_ap(S[:], [(STORY, nb), (1, STORY)], k * SLOTS + off * STORY)

                def t0v(off=0, nb=BSH):
                    return _mk_ap(t0[:], [(STORY, nb), (1, STORY)], off * STORY)

                def t0f(off, n):
                    return _mk_ap(t0[:], [(1, n)], off)

                ta_b = _mk_ap(t_tat[:], [(0, BSH), (1, STORY)])
                tc_bh = _mk_ap(t_tct[:], [(0, BSH // 2), (1, STORY)])
                u_b = _mk_ap(u[:], [(1, BSH), (0, STORY)])
                HB = SLOTS // 2  # 400

                for k in range(HOPS):
                    # t0 = (scale * S[k] + TA bcast) * u bcast
                    nc.vector.scalar_tensor_tensor(
                        t0v(), smv(k), scale, ta_b,
                        mybir.AluOpType.mult, mybir.AluOpType.add)
                    nc.vector.tensor_mul(t0v(), t0v(), u_b)
                    # partition-reduce -> logits [1, 16, 50] (two 400-wide psum
                    # banks); exp straight off PSUM. No max-subtract: |logit|
                    # <= ~40 for this model scale, exp stays inside f32.
                    for h in range(2):
                        pl = ppool.tile([1, HB], f32, tag="pl", space="PSUM")
                        nc.tensor.matmul(
                            pl[:], lhsT=ones_col[:], rhs=t0f(h * HB, HB),
                            start=True, stop=True)
                        nc.scalar.activation(
                            _mk_ap(lg[:], [(1, HB)], h * HB), pl[:],
                            mybir.ActivationFunctionType.Exp)
                    nc.vector.tensor_reduce(red2[:], lg[:], mybir.AxisListType.X, mybir.AluOpType.add)
                    nc.vector.reciprocal(red2[:], red2[:])
                    red2_b = _mk_ap(red2[:], [(1, BSH), (0, STORY)])
                    nc.vector.tensor_mul(pe_sb[0:1, :, :], lg[:], red2_b)
                    # broadcast p to all partitions; then t0 = (SM[k+1] + TC bcast) * p
                    for h in range(2):
                        pb = ppool.tile([128, HB], f32, tag="pb", space="PSUM")
                        nc.tensor.matmul(
                            pb[:], lhsT=e0row[:],
                            rhs=_mk_ap(pe_sb[:], [(1, HB)], h * HB),
                            start=True, stop=True)
                        pb3 = _mk_ap(pb[:], [(STORY, BSH // 2), (1, STORY)])
                        nc.vector.scalar_tensor_tensor(
                            t0v(h * (BSH // 2), BSH // 2),
                            smv(k + 1, h * (BSH // 2), BSH // 2), scale, tc_bh,
                            mybir.AluOpType.mult, mybir.AluOpType.add)
                        nc.vector.tensor_mul(
                            t0v(h * (BSH // 2), BSH // 2),
                            t0v(h * (BSH // 2), BSH // 2), pb3)
                    # u += sum_s t0
                    nc.vector.tensor_reduce(red_u[:], t0v(), mybir.AxisListType.X, mybir.AluOpType.add)
                    nc.vector.tensor_add(u[:], u[:], red_u[:])

                # ---- projection: out[v, b] = sum_e A3[v, e] * u[e, b]
                # Per-block PSUM -> SBUF -> DRAM so stores overlap later matmuls.
                u16 = hpool.tile([128, BSH], f16, tag="u16")
                nc.vector.tensor_copy(u16[:], u[:])
                with (
                    tc.tile_pool(name="a3pool", bufs=3) as apool,
                    tc.tile_pool(name="opool", bufs=3) as opool,
                ):
                    CPL = 32  # vocab chunks (of 128) per a3t load; 32*16 = 512 f32 = 1 PSUM bank
                    for blk in range(NVC // CPL + (1 if NVC % CPL else 0)):
                        n_in_blk = min(CPL, NVC - blk * CPL)
                        a3c = apool.tile([128, CPL * 128], f16, tag="a3c")
                        nc.sync.dma_start(
                            a3c[:, :n_in_blk * 128],
                            a3t[:, blk * CPL * 128: blk * CPL * 128 + n_in_blk * 128])
                        po = ppool.tile([128, CPL * BSH], f32, tag="po", space="PSUM")
                        for w in range(n_in_blk):
                            nc.tensor.matmul(
                                po[:, w * BSH:(w + 1) * BSH],
                                lhsT=a3c[:, w * 128:(w + 1) * 128],
                                rhs=u16[:], start=True, stop=True)
                        ob = opool.tile([128, CPL * BSH], f32, tag="ob")
                        nc.vector.tensor_copy(
                            ob[:, :n_in_blk * BSH], po[:, :n_in_blk * BSH])
                        c0 = blk * CPL * BSH
                        nc.sync.dma_start(
                            out[:, c0:c0 + n_in_blk * BSH],
                            ob[:, :n_in_blk * BSH])

    nc.compile()
    nc._gather_names = gather_names
    return nc


def _build_tuned(scale):
    """Two-pass build: learn scheduled SWDGE order, then align queue_num with
    Tile's DMASW-lane round-robin (lane n%8 must always see queue n%4)."""
    nc0 = _build(scale)
    sched = []
    for b in nc0.main_func.blocks:
        for i in b.instructions:
            if type(i).__name__ == "InstDMAGatherAnt":
                sched.append(i.name)
    return nc0  # multi-queue SWDGE corrupts concurrent transpose gathers on HW


def _prep_inputs(x, q, A, TA, TC):
    """Host-side marshalling: megatable, A3^T, per-core index lists."""
    x = np.asarray(x).astype(np.int64)
    q = np.asarray(q).astype(np.int64)
    A = np.asarray(A, dtype=np.float32)
    TA = np.asarray(TA, dtype=np.float32)
    TC = np.asarray(TC, dtype=np.float32)

    s = float(np.abs(A).max())
    scale = s / 32767.0
    Aq = np.round(A / s * 32767.0).astype(np.int16)
    # megarow v: [A0, A1, A2, A3] int16 (1024 B)
    mega = np.zeros((NROWS, 512), dtype=np.int16)
    for k in range(4):
        mega[:VOCAB, k * 128:(k + 1) * 128] = Aq[k]

    a3t = np.zeros((128, VPAD), dtype=np.float16)
    a3t[:, :VOCAB] = A[3].astype(np.float16).T

    tat = np.ascontiguousarray(TA[0].T)  # [128, 50]
    tct = np.ascontiguousarray(TC[0].T)

    in_maps = []
    for c in range(NCORES):
        xs = x[c * BSH:(c + 1) * BSH].reshape(-1)        # [16000] slot-major
        qs = q[c * BSH:(c + 1) * BSH].reshape(-1)        # [320]
        # Per-core row permutation: rows this core touches (plus the zero
        # padding row 0) are moved to the front, so every remapped index is
        # < 16321 and one int16 gather pass covers everything.
        used = np.unique(np.concatenate([[0], xs, qs]))  # sorted, 0 first
        nu = used.shape[0]
        order = np.empty(NROWS, dtype=np.int64)
        order[:nu] = used
        mask = np.ones(NROWS, dtype=bool)
        mask[used] = False
        order[nu:] = np.nonzero(mask)[0]
        pos = np.empty(NROWS, dtype=np.int64)
        pos[order] = np.arange(NROWS)
        mega_c = mega[order]
        idx = pos[xs]
        qp = np.zeros(BSH * QPAD, dtype=np.int64)        # pad -> row 0 (zeros)
        qp2 = qp.reshape(BSH, QPAD)
        qp2[:, :QLEN] = pos[qs].reshape(BSH, QLEN)
        in_maps.append({
            "mega": mega_c, "a3t": a3t, "tat": tat, "tct": tct,
            "ilo": _wrap_idxs(idx), "iqlo": _wrap_idxs(qp),
        })
    return in_maps, scale


def kernel(x, q, A, TA, TC):
    import os
    from concourse.bass_utils import run_bass_kernel_spmd

    in_maps, scale = _prep_inputs(x, q, A, TA, TC)
    if _cache.get("scale") != scale:
        _cache["nc"] = _build_tuned(scale)
        _cache["scale"] = scale
    nc = _cache["nc"]
    trace = bool(int(os.environ.get("MEMNN_TRACE", "0")))
    res = run_bass_kernel_spmd(nc, in_maps, list(range(NCORES)), trace=trace)
    if trace:
        _cache["exec_time_ns"] = res.exec_time_ns
        _cache["mean_exec_time_ns"] = res.mean_exec_time_ns
        _cache["results"] = res

    outs = []
    for c in range(NCORES):
        oc = res.results[c]["outp"].reshape(128, NVC, BSH)
        full = oc.transpose(1, 0, 2).reshape(VPAD, BSH)   # [50048, 16]
        outs.append(full[:VOCAB].T)                       # [16, 50000]
    return np.concatenate(outs, axis=0).astype(np.float32)



# revision 25
# speedup vs baseline: 1.1265x; 1.1265x over previous
"""MemNN layer kernel for 8 Trainium2 NeuronCores.

Strategy (batch-sharded, 16 batches/core):
- The 4 embedding tables are interleaved into one "megatable" whose row v is
  [A0hi|A0lo|A1hi|A1lo|A2hi|A2lo|A3hi|A3lo] (8 x 128 fp16 = 2048 B), where
  hi = fp16(A), lo = fp16(A - hi).  One dma_gather fetches all 4 tables for a
  token at f32-exact precision (hi+lo), at the same GpSimd descriptor-gen cost
  as a single-table gather (cost is per-index, not per-byte).
- dma_gather in transpose mode lands embeddings with embd on partitions:
  G[p, slice, i] = megarow(idx_i)[slice*128+p].  int16 gather indices only
  reach 32767, so two passes per chunk: lo pass (rows < 32768, sentinel row 0
  which is all-zero padding) and hi pass (rows >= 32768 rebased, sentinel ->
  appended all-zero row 50000).
- 20-token sentence sums: DVE tensor_reduce over a 5-D AP that folds the
  lo/hi passes and the 20 tokens in one op -> S[128e, 8slices, 800slots] f32.
- Hops run in embd-on-partition layout: logits via elementwise mul + ones-
  matmul partition reduce, softmax on [1, 800], p broadcast via e0-selector
  matmul, weighted c-sum via DVE reduce.
- Final projection out[v, b] = sum_e A3[v, e] u[e, b]: A3^T fp16 is streamed
  from DRAM (pre-transposed on host) as PE lhsT per 128-vocab chunk, rhs =
  fp16(u); PSUM -> SBUF -> one contiguous store in [128, 391, 16] layout that
  the host rearranges.
"""

import numpy as np

HOPS = 3
VOCAB = 50000
EMBD = 128
BS = 128
STORY = 50
SENT = 20
QLEN = 20
NCORES = 8
BSH = BS // NCORES          # 16 batches per core
SLOTS = BSH * STORY         # 800 (b, s) slots per core
SPLIT = 32768               # int16-reachable rows per gather base
ZROW = VOCAB                # appended all-zero megatable row
HI_SENT = ZROW - SPLIT      # 17232: hi-pass sentinel (-> zero row)
NROWS = VOCAB + 1           # 50001
CHUNKS = [1280] * 12 + [640]  # %128==0, %20==0; 322 rx descs/lane so 3 overlap in the 1024 ring
GBUFS = 3                     # gather tile-pool depth
PREP_NQ = 0                   # >0: prepare_only desc-gen on PREP_NQ queues, serialized triggers
QPAD = 24                   # per-batch query tokens padded 20 -> 24
QIDX = BSH * QPAD           # 384 (%128 == 0)
VPAD = 50048                # vocab padded to 391*128 for projection
NVC = VPAD // 128           # 391 projection chunks

_cache = {}


def _wrap_idxs(lst):
    """int16 gather index layout: [128, n/16]; position i -> [i%16, i//16], tiled 8x."""
    a = np.asarray(lst).astype(np.int16).reshape(-1, 16).T.copy()
    return np.tile(a, (8, 1))


def _mk_ap(base_ap, dims, extra_offset_elems=0):
    """AP with the partition pair of base_ap and given free (stride, count) pairs."""
    import concourse.bass as bass
    ap = [tuple(base_ap.ap[0])] + [tuple(d) for d in dims]
    return bass.AP(base_ap.tensor, base_ap.offset + extra_offset_elems, ap)


def _build(scale, qmap=None):
    import concourse.tile as tile
    from concourse import bacc, mybir

    f32 = mybir.dt.float32
    i32 = mybir.dt.int32
    f16 = mybir.dt.float16
    i16 = mybir.dt.int16

    nq = PREP_NQ if PREP_NQ else (1 + max(qmap.values()) if qmap else 1)
    nc = bacc.Bacc("TRN2", target_bir_lowering=False, debug=False,
                   num_swdge_queues=nq)
    gather_names = []
    _gi = [0]

    def _gq():
        i = _gi[0]
        _gi[0] += 1
        return qmap.get(i, 0) if qmap else 0

    mega = nc.dram_tensor("mega", [NROWS, 512], i16, kind="ExternalInput")
    a3t = nc.dram_tensor("a3t", [128, VPAD], f16, kind="ExternalInput")
    ilo = nc.dram_tensor("ilo", [128, SLOTS * SENT // 16], i16, kind="ExternalInput")
    iqlo = nc.dram_tensor("iqlo", [128, QIDX // 16], i16, kind="ExternalInput")
    tat = nc.dram_tensor("tat", [128, STORY], f32, kind="ExternalInput")
    tct = nc.dram_tensor("tct", [128, STORY], f32, kind="ExternalInput")
    out = nc.dram_tensor("outp", [128, NVC * BSH], f32, kind="ExternalOutput")

    with tile.TileContext(nc) as tc:
        with (
            tc.tile_pool(name="consts", bufs=1) as cpool,
            tc.tile_pool(name="sacc", bufs=1) as spool,
        ):
            # ---- index loads first: they gate the first gather
            t_ilo = cpool.tile([128, SLOTS * SENT // 16], i16, tag="ilo")
            nc.sync.dma_start(t_ilo[:], ilo[:])
            t_iqlo = cpool.tile([128, QIDX // 16], i16, tag="iqlo")
            nc.sync.dma_start(t_iqlo[:], iqlo[:])

            t_tat = cpool.tile([128, STORY], f32, tag="tat")
            nc.sync.dma_start(t_tat[:], tat[:])
            t_tct = cpool.tile([128, STORY], f32, tag="tct")
            nc.sync.dma_start(t_tct[:], tct[:])
            # Full A3^T prefetch: rides the ~50% DMA idle of the desc-gen
            # bound gather phase so the projection is purely PE-bound.
            a3all = cpool.tile([128, VPAD], f16, tag="a3all")
            nc.sync.dma_start(a3all[:], a3t[:])
            ones_col = cpool.tile([128, 1], f32, tag="ones_col")  # lhsT for partition sum
            nc.vector.memset(ones_col[:], 1.0)
            e0row = cpool.tile([128, 128], f16, tag="e0row")      # lhsT for p broadcast
            nc.vector.memset(e0row[:], 0.0)
            nc.vector.memset(e0row[0:1, :], 1.0)

            # ---- S accumulator [128, 8 slices, 800 slots] f32
            S = spool.tile([128, 4, SLOTS], i32, tag="S")
            uq = spool.tile([128, 4, BSH], f32, tag="uq")  # query-sum per slice

            # ---- gather + reduce phase (int16 megatable, 1024 B rows)
            # int32 accumulation of int16 values is exact; scale applied later
            low_prec = nc.allow_low_precision(reason="int32 accumulation of int16 is exact")
            low_prec.__enter__()
            # PREP_NQ mode: all desc-gen preps are emitted first (they run
            # concurrently on the Q7s, one ring per queue), then count=1
            # triggers fire the transfers strictly in chunk order -- each
            # trigger waits on its own prep (psem) and on the previous
            # chunk's DMA completion (gsem of that chunk's queue) so
            # transpose-gather DMA streams never interleave across queues.
            # Consumer reduces carry explicit DMA-completion waits (Tile's
            # auto-wiring for prepare_only gathers proved wrong on HW).
            ngath = len(CHUNKS) + 1
            all_chunks = [QIDX] + list(CHUNKS)  # query first: its reduce is tiny
            if PREP_NQ:
                gq_of = [i % PREP_NQ for i in range(ngath)]
                qpos = []  # index of chunk within its queue
                _cnt = [0] * PREP_NQ
                for i in range(ngath):
                    qpos.append(_cnt[gq_of[i]])
                    _cnt[gq_of[i]] += 1
                gsems = [nc.alloc_semaphore(f"gdma{q}") for q in range(PREP_NQ)]
                psems = [nc.alloc_semaphore(f"gprep{q}") for q in range(PREP_NQ)]

            with tc.tile_pool(name="gath", bufs=(1 if PREP_NQ else GBUFS)) as gpool:
                gq_lo = gpool.tile([128, 4, QIDX], i16, tag="gq_lo")
                tiles = [gq_lo]
                for ci, ch in enumerate(CHUNKS):
                    g_lo = gpool.tile([128, 4, ch], i16, tag=f"g_lo{ci if PREP_NQ else ''}")
                    tiles.append(g_lo)

                def idx_ap_of(ci):
                    if ci == 0:
                        return t_iqlo[:]
                    pos = sum(CHUNKS[:ci - 1])
                    return t_ilo[:, pos // 16:(pos + CHUNKS[ci - 1]) // 16]

                if PREP_NQ:
                    for ci, ch in enumerate(all_chunks):
                        qn = gq_of[ci]
                        prep = nc.gpsimd.dma_gather(
                            tiles[ci][:], mega[:], idx_ap_of(ci), ch, ch, 512,
                            transpose=True, single_packet=False,
                            queue_num=qn, prepare_only=True, sem=gsems[qn])
                        prep.then_inc(psems[qn], 1)
                        gather_names.append(prep.ins.name)
                    for ci in range(ngath):
                        qn = gq_of[ci]
                        if ci > 0:
                            pq = gq_of[ci - 1]
                            nc.gpsimd.wait_ge(gsems[pq], 16 * (qpos[ci - 1] + 1))
                        tr = nc.gpsimd.trigger_dma(count=1, queue_num=qn)
                        tr._wait_ge(psems[qn], qpos[ci] + 1)
                else:
                    for ci, ch in enumerate(all_chunks):
                        gather_names.append(nc.gpsimd.dma_gather(
                            tiles[ci][:], mega[:], idx_ap_of(ci), ch, ch, 512,
                            transpose=True, single_packet=False,
                            queue_num=_gq()).ins.name)

                # query-token sums: [128, 4, 16, 24] reduce X(24) -> uq
                q_lo_in = _mk_ap(gq_lo[:], [(QIDX, 4), (QPAD, BSH), (1, QPAD)])
                if PREP_NQ:
                    nc.vector.wait_ge(gsems[gq_of[0]], 16 * (qpos[0] + 1))
                nc.vector.tensor_reduce(
                    uq[:], q_lo_in, mybir.AxisListType.X, mybir.AluOpType.add)

                pos = 0
                for ci, ch in enumerate(CHUNKS):
                    nslot = ch // SENT
                    s0 = pos // SENT
                    # DVE reduce over tokens [128, 4, nslot, 20] -> int32 (exact)
                    red_lo = _mk_ap(tiles[ci + 1][:], [(ch, 4), (SENT, nslot), (1, SENT)])
                    if PREP_NQ:
                        nc.vector.wait_ge(gsems[gq_of[ci + 1]], 16 * (qpos[ci + 1] + 1))
                    nc.vector.tensor_reduce(
                        S[:, :, s0:s0 + nslot], red_lo,
                        mybir.AxisListType.X, mybir.AluOpType.add)
                    pos += ch

            low_prec.__exit__(None, None, None)
            with (
                tc.tile_pool(name="hopp", bufs=1) as hpool,
                tc.tile_pool(name="psum", bufs=2, space="PSUM") as ppool,
            ):
                # u0 = scale * (q-sum of table 0)
                u = hpool.tile([128, BSH], f32, tag="u")
                nc.vector.tensor_scalar_mul(u[:], uq[:, 0, :], scale)

                t0 = hpool.tile([128, BSH, STORY], f32, tag="t0")
                pe_sb = hpool.tile([128, BSH, STORY], f16, tag="pe_sb")
                nc.vector.memset(pe_sb[:], 0.0)
                lg = hpool.tile([1, BSH, STORY], f32, tag="lg")
                red2 = hpool.tile([1, BSH], f32, tag="red2")
                red_u = hpool.tile([128, BSH], f32, tag="redu")

                def smv(k, off=0, nb=BSH):
                    return _mk_ap(S[:], [(STORY, nb), (1, STORY)], k * SLOTS + off * STORY)

                def t0v(off=0, nb=BSH):
                    return _mk_ap(t0[:], [(STORY, nb), (1, STORY)], off * STORY)

                def t0f(off, n):
                    return _mk_ap(t0[:], [(1, n)], off)

                ta_b = _mk_ap(t_tat[:], [(0, BSH), (1, STORY)])
                tc_bh = _mk_ap(t_tct[:], [(0, BSH // 2), (1, STORY)])
                u_b = _mk_ap(u[:], [(1, BSH), (0, STORY)])
                HB = SLOTS // 2  # 400

                for k in range(HOPS):
                    # t0 = (scale * S[k] + TA bcast) * u bcast
                    nc.vector.scalar_tensor_tensor(
                        t0v(), smv(k), scale, ta_b,
                        mybir.AluOpType.mult, mybir.AluOpType.add)
                    nc.vector.tensor_mul(t0v(), t0v(), u_b)
                    # partition-reduce -> logits [1, 16, 50] (two 400-wide psum
                    # banks); exp straight off PSUM. No max-subtract: |logit|
                    # <= ~40 for this model scale, exp stays inside f32.
                    for h in range(2):
                        pl = ppool.tile([1, HB], f32, tag="pl", space="PSUM")
                        nc.tensor.matmul(
                            pl[:], lhsT=ones_col[:], rhs=t0f(h * HB, HB),
                            start=True, stop=True)
                        nc.scalar.activation(
                            _mk_ap(lg[:], [(1, HB)], h * HB), pl[:],
                            mybir.ActivationFunctionType.Exp)
                    nc.vector.tensor_reduce(red2[:], lg[:], mybir.AxisListType.X, mybir.AluOpType.add)
                    nc.vector.reciprocal(red2[:], red2[:])
                    red2_b = _mk_ap(red2[:], [(1, BSH), (0, STORY)])
                    nc.vector.tensor_mul(pe_sb[0:1, :, :], lg[:], red2_b)
                    # broadcast p to all partitions; then t0 = (SM[k+1] + TC bcast) * p
                    for h in range(2):
                        pb = ppool.tile([128, HB], f32, tag="pb", space="PSUM")
                        nc.tensor.matmul(
                            pb[:], lhsT=e0row[:],
                            rhs=_mk_ap(pe_sb[:], [(1, HB)], h * HB),
                            start=True, stop=True)
                        pb3 = _mk_ap(pb[:], [(STORY, BSH // 2), (1, STORY)])
                        nc.vector.scalar_tensor_tensor(
                            t0v(h * (BSH // 2), BSH // 2),
                            smv(k + 1, h * (BSH // 2), BSH // 2), scale, tc_bh,
                            mybir.AluOpType.mult, mybir.AluOpType.add)
                        nc.vector.tensor_mul(
                            t0v(h * (BSH // 2), BSH // 2),
                            t0v(h * (BSH // 2), BSH // 2), pb3)
                    # u += sum_s t0
                    nc.vector.tensor_reduce(red_u[:], t0v(), mybir.AxisListType.X, mybir.AluOpType.add)
                    nc.vector.tensor_add(u[:], u[:], red_u[:])

                # ---- projection: out[v, b] = sum_e A3[v, e] * u[e, b]
                # Per-block PSUM -> SBUF -> DRAM so stores overlap later matmuls.
                u16 = hpool.tile([128, BSH], f16, tag="u16")
                nc.vector.tensor_copy(u16[:], u[:])
                with tc.tile_pool(name="opool", bufs=3) as opool:
                    CPL = 32  # vocab chunks (of 128) per block; 32*16 = 512 f32 = 1 PSUM bank
                    for blk in range(NVC // CPL + (1 if NVC % CPL else 0)):
                        n_in_blk = min(CPL, NVC - blk * CPL)
                        b0 = blk * CPL * 128
                        po = ppool.tile([128, CPL * BSH], f32, tag="po", space="PSUM")
                        for w in range(n_in_blk):
                            nc.tensor.matmul(
                                po[:, w * BSH:(w + 1) * BSH],
                                lhsT=a3all[:, b0 + w * 128:b0 + (w + 1) * 128],
                                rhs=u16[:], start=True, stop=True)
                        ob = opool.tile([128, CPL * BSH], f32, tag="ob")
                        nc.vector.tensor_copy(
                            ob[:, :n_in_blk * BSH], po[:, :n_in_blk * BSH])
                        c0 = blk * CPL * BSH
                        nc.sync.dma_start(
                            out[:, c0:c0 + n_in_blk * BSH],
                            ob[:, :n_in_blk * BSH])

    nc.compile()
    nc._gather_names = gather_names
    return nc


def _build_tuned(scale):
    """Two-pass build: learn scheduled SWDGE order, then align queue_num with
    Tile's DMASW-lane round-robin (lane n%8 must always see queue n%4)."""
    nc0 = _build(scale)
    sched = []
    for b in nc0.main_func.blocks:
        for i in b.instructions:
            if type(i).__name__ == "InstDMAGatherAnt":
                sched.append(i.name)
    return nc0  # multi-queue SWDGE corrupts concurrent transpose gathers on HW


def _prep_inputs(x, q, A, TA, TC):
    """Host-side marshalling: megatable, A3^T, per-core index lists."""
    x = np.asarray(x).astype(np.int64)
    q = np.asarray(q).astype(np.int64)
    A = np.asarray(A, dtype=np.float32)
    TA = np.asarray(TA, dtype=np.float32)
    TC = np.asarray(TC, dtype=np.float32)

    s = float(np.abs(A).max())
    scale = s / 32767.0
    Aq = np.round(A / s * 32767.0).astype(np.int16)
    # megarow v: [A0, A1, A2, A3] int16 (1024 B)
    mega = np.zeros((NROWS, 512), dtype=np.int16)
    for k in range(4):
        mega[:VOCAB, k * 128:(k + 1) * 128] = Aq[k]

    a3t = np.zeros((128, VPAD), dtype=np.float16)
    a3t[:, :VOCAB] = A[3].astype(np.float16).T

    tat = np.ascontiguousarray(TA[0].T)  # [128, 50]
    tct = np.ascontiguousarray(TC[0].T)

    in_maps = []
    for c in range(NCORES):
        xs = x[c * BSH:(c + 1) * BSH].reshape(-1)        # [16000] slot-major
        qs = q[c * BSH:(c + 1) * BSH].reshape(-1)        # [320]
        # Per-core row permutation: rows this core touches (plus the zero
        # padding row 0) are moved to the front, so every remapped index is
        # < 16321 and one int16 gather pass covers everything.
        used = np.unique(np.concatenate([[0], xs, qs]))  # sorted, 0 first
        nu = used.shape[0]
        order = np.empty(NROWS, dtype=np.int64)
        order[:nu] = used
        mask = np.ones(NROWS, dtype=bool)
        mask[used] = False
        order[nu:] = np.nonzero(mask)[0]
        pos = np.empty(NROWS, dtype=np.int64)
        pos[order] = np.arange(NROWS)
        mega_c = mega[order]
        idx = pos[xs]
        qp = np.zeros(BSH * QPAD, dtype=np.int64)        # pad -> row 0 (zeros)
        qp2 = qp.reshape(BSH, QPAD)
        qp2[:, :QLEN] = pos[qs].reshape(BSH, QLEN)
        in_maps.append({
            "mega": mega_c, "a3t": a3t, "tat": tat, "tct": tct,
            "ilo": _wrap_idxs(idx), "iqlo": _wrap_idxs(qp),
        })
    return in_maps, scale


def kernel(x, q, A, TA, TC):
    import os
    from concourse.bass_utils import run_bass_kernel_spmd

    in_maps, scale = _prep_inputs(x, q, A, TA, TC)
    if _cache.get("scale") != scale:
        _cache["nc"] = _build_tuned(scale)
        _cache["scale"] = scale
    nc = _cache["nc"]
    trace = bool(int(os.environ.get("MEMNN_TRACE", "0")))
    res = run_bass_kernel_spmd(nc, in_maps, list(range(NCORES)), trace=trace)
    if trace:
        _cache["exec_time_ns"] = res.exec_time_ns
        _cache["mean_exec_time_ns"] = res.mean_exec_time_ns
        _cache["results"] = res

    outs = []
    for c in range(NCORES):
        oc = res.results[c]["outp"].reshape(128, NVC, BSH)
        full = oc.transpose(1, 0, 2).reshape(VPAD, BSH)   # [50048, 16]
        outs.append(full[:VOCAB].T)                       # [16, 50000]
    return np.concatenate(outs, axis=0).astype(np.float32)

